# revision 1
# baseline (speedup 1.0000x reference)
"""GCN encoder (3x GCNConv + mean-pool + MLP) as an 8-core Trainium2 Bass kernel.

Sharding: nodes/edges partitioned by destination-node owner (8 shards).
Per layer: per-edge source features are gathered from a per-core DRAM table
(fp16) with dma_gather, scaled+scattered into per-destination sums via a
PE matmul against a one-hot selection matrix built on DVE, then the layer
weight matmul + bias + ReLU produces this core's shard of the next layer's
features, which an AllGather collective replicates into every core's table.
Mean-pool is a matmul against a per-graph one-hot (scaled by 1/count),
AllReduce-summed across cores; the tiny MLP is computed replicated.
"""

import numpy as np

NCORES = 8
F = 128            # hidden width (all layers padded to this)
G = 256            # number of graphs
NH = 512           # MLP hidden
NO = 256           # MLP out
CH = 128           # edges per chunk
BATCH_CH = 32      # chunks per dma_gather batch
WINW = 256         # dst nodes per PSUM accumulation window

_cache = {}


def _host_prep(x, edge_index, batch, W0, b0, W1, b1, W2, b2, Wm1, bm1, Wm2, bm2):
    N = x.shape[0]
    FI = x.shape[1]
    SH = -(-N // (NCORES * 128)) * 128      # shard size (nodes), 128-multiple
    NP = SH * NCORES
    TILES = SH // 128
    NWIN = -(-SH // WINW)
    LO = min(32768, NP)
    HI = NP - LO

    src = np.concatenate([edge_index[0], np.arange(N, dtype=np.int64)])
    dst = np.concatenate([edge_index[1], np.arange(N, dtype=np.int64)])
    deg = np.bincount(dst, minlength=N).astype(np.float32)
    dis = np.where(deg > 0, 1.0 / np.sqrt(np.maximum(deg, 1.0)), 0.0).astype(np.float32)
    norm = dis[src] * dis[dst]

    xpad = np.zeros((NP, F), dtype=np.float16)
    xpad[:N, :FI] = x.astype(np.float16)

    # per-core edge selection, ordered by (window, class, dst)
    per_core = []
    for c in range(NCORES):
        base = c * SH
        sel = (dst >= base) & (dst < base + SH)
        es = src[sel].astype(np.int64)
        ed = (dst[sel] - base).astype(np.int64)
        en = norm[sel]
        cl = (es >= LO).astype(np.int64)
        wi = ed // WINW
        order = np.lexsort((ed, cl, wi))
        per_core.append((es[order], ed[order], en[order], cl[order], wi[order]))

    # chunk counts per (window, class), equalized across cores
    nch = np.zeros((NWIN, 2), dtype=np.int64)
    counts = np.zeros((NCORES, NWIN, 2), dtype=np.int64)
    for c in range(NCORES):
        _, _, _, cl, wi = per_core[c]
        for cls in (0, 1):
            cnt = np.bincount(wi[cl == cls], minlength=NWIN)
            counts[c, :, cls] = cnt
    nch = -(-counts.max(axis=0) // CH)  # [NWIN, 2] chunks
    nch_cls = nch.sum(axis=0)          # total chunks per class
    ncht = int(nch.sum())

    # shared program schedule: windows -> list of (cls, cid); meta col = global g
    schedule = []
    cid_ctr = [0, 0]
    for w in range(NWIN):
        lst = []
        for cls in (0, 1):
            for _ in range(int(nch[w, cls])):
                lst.append((cls, cid_ctr[cls]))
                cid_ctr[cls] += 1
        schedule.append(lst)

    # per-core streams
    idx_streams = [[], []]   # per class: list over cores of int16 arrays
    metas = []
    for c in range(NCORES):
        es, ed, en, cl, wi = per_core[c]
        idx_parts = [[], []]
        meta = np.zeros((128, 2 * ncht), dtype=np.float32)
        g = 0
        pos = 0
        # edges are sorted (win, cls, dst); walk groups in the same order
        for w in range(NWIN):
            for cls in (0, 1):
                n_e = int(counts[c, w, cls])
                tot = int(nch[w, cls]) * CH
                ge, gd, gn = es[pos:pos + n_e], ed[pos:pos + n_e], en[pos:pos + n_e]
                pos += n_e
                pad = tot - n_e
                iv = ge - (LO if cls else 0)
                iv = np.concatenate([iv, np.zeros(pad, np.int64)])
                dl = np.concatenate([gd - w * WINW, np.zeros(pad, np.int64)])
                nr = np.concatenate([gn, np.zeros(pad, np.float32)])
                idx_parts[cls].append(iv.astype(np.int16))
                for k in range(tot // CH):
                    meta[:, 2 * g] = dl[k * CH:(k + 1) * CH].astype(np.float32)
                    meta[:, 2 * g + 1] = nr[k * CH:(k + 1) * CH].astype(np.float32)
                    g += 1
        assert g == ncht
        for cls in (0, 1):
            arr = (np.concatenate(idx_parts[cls]) if idx_parts[cls]
                   else np.zeros(0, np.int16))
            assert arr.size == nch_cls[cls] * CH
            if arr.size:
                wrapped = np.tile(arr.reshape(-1, 16).T, (8, 1))
            else:
                wrapped = np.zeros((128, 8), np.int16)  # dummy
            idx_streams[cls].append(np.ascontiguousarray(wrapped))
        metas.append(meta)

    # pooling helpers
    cnt = np.bincount(batch.astype(np.int64), minlength=G).astype(np.float32)
    invc_all = (1.0 / np.maximum(cnt, 1.0))[batch.astype(np.int64)]
    bcols, invcs = [], []
    for c in range(NCORES):
        sl = slice(c * SH, min((c + 1) * SH, N))
        b_sh = np.zeros(SH, np.float32)
        i_sh = np.zeros(SH, np.float32)
        nreal = max(0, min((c + 1) * SH, N) - c * SH)
        if nreal > 0:
            b_sh[:nreal] = batch[sl].astype(np.float32)
            i_sh[:nreal] = invc_all[sl].astype(np.float32)
        bcols.append(np.ascontiguousarray(b_sh.reshape(TILES, 128).T))  # [128,TILES]
        invcs.append(np.ascontiguousarray(i_sh.reshape(TILES, 128).T))

    W0p = np.zeros((F, F), np.float16)
    W0p[:FI] = W0.astype(np.float16)
    consts = {
        "w0": W0p, "w1": W1.astype(np.float16), "w2": W2.astype(np.float16),
        "wm1": Wm1.astype(np.float16), "wm2": Wm2.astype(np.float16),
        "b0r": np.tile(b0.astype(np.float32)[None, :], (128, 1)),
        "b1r": np.tile(b1.astype(np.float32)[None, :], (128, 1)),
        "b2r": np.tile(b2.astype(np.float32)[None, :], (128, 1)),
        "bm1c": np.ascontiguousarray(bm1.astype(np.float32).reshape(4, 128).T),
        "bm2r": np.tile(bm2.astype(np.float32)[None, :], (128, 1)),
        "iota": np.tile(np.arange(G, dtype=np.float16)[None, :], (128, 1)),
    }
    in_maps = []
    for c in range(NCORES):
        m = dict(consts)
        m["xtab"] = xpad
        m["idxlo"] = idx_streams[0][c]
        m["idxhi"] = idx_streams[1][c]
        m["meta"] = metas[c]
        m["bcol"] = bcols[c]
        m["invc"] = invcs[c]
        in_maps.append(m)

    geom = dict(N=N, NP=NP, SH=SH, TILES=TILES, NWIN=NWIN, LO=LO, HI=HI,
                nch=nch, nch_cls=[int(v) for v in nch_cls], ncht=ncht,
                schedule=schedule)
    return geom, in_maps


class _SkipRest(Exception):
    pass


def _build_bass(geom, variant="full"):
    import concourse.bass as bass
    import concourse.tile as tile
    from concourse import bacc, mybir

    f16, f32, i16 = mybir.dt.float16, mybir.dt.float32, mybir.dt.int16
    NP, SH, TILES, NWIN = geom["NP"], geom["SH"], geom["TILES"], geom["NWIN"]
    LO, HI = geom["LO"], geom["HI"]
    nch, nch_cls, ncht = geom["nch"], geom["nch_cls"], geom["ncht"]
    schedule = geom["schedule"]

    nc = bacc.Bacc("TRN2", target_bir_lowering=False, debug=False,
                   num_devices=NCORES)

    xtab = nc.dram_tensor("xtab", [NP, F], f16, kind="ExternalInput")
    idxlo = nc.dram_tensor("idxlo", [128, max(nch_cls[0] * 8, 8)], i16, kind="ExternalInput")
    idxhi = nc.dram_tensor("idxhi", [128, max(nch_cls[1] * 8, 8)], i16, kind="ExternalInput")
    meta = nc.dram_tensor("meta", [128, 2 * ncht], f32, kind="ExternalInput")
    w_in = {n: nc.dram_tensor(n, [F, F], f16, kind="ExternalInput")
            for n in ("w0", "w1", "w2")}
    wm1 = nc.dram_tensor("wm1", [F, NH], f16, kind="ExternalInput")
    wm2 = nc.dram_tensor("wm2", [NH, NO], f16, kind="ExternalInput")
    b_in = {n: nc.dram_tensor(n, [128, F], f32, kind="ExternalInput")
            for n in ("b0r", "b1r", "b2r")}
    bm1c = nc.dram_tensor("bm1c", [128, 4], f32, kind="ExternalInput")
    bm2r = nc.dram_tensor("bm2r", [128, NO], f32, kind="ExternalInput")
    iota = nc.dram_tensor("iota", [128, G], f16, kind="ExternalInput")
    bcol = nc.dram_tensor("bcol", [128, TILES], f32, kind="ExternalInput")
    invc = nc.dram_tensor("invc", [128, TILES], f32, kind="ExternalInput")
    out = nc.dram_tensor("out", [G, NO], f32, kind="ExternalOutput")

    shard_d = nc.dram_tensor("shard_d", [SH, F], f16)
    tabn = nc.dram_tensor("tabn", [NP, F], f16, addr_space="Shared")
    gt_in = nc.dram_tensor("gt_in", [128, G], f32)
    gt_out = nc.dram_tensor("gt_out", [128, G], f32, addr_space="Shared")

    shb = nc.alloc_sbuf_tensor("shb", [128, TILES * F], f16)

    import contextlib
    with tile.TileContext(nc) as tc:
        with (
            contextlib.suppress(_SkipRest),
            tc.tile_pool(name="res", bufs=1) as res,
            tc.tile_pool(name="msg", bufs=3) as msgp,
            tc.tile_pool(name="sp", bufs=4) as sp,
            tc.tile_pool(name="agg", bufs=2) as aggp,
            tc.tile_pool(name="tmp", bufs=2) as tmpp,
            tc.tile_pool(name="wps", bufs=2, space="PSUM") as wps,
            tc.tile_pool(name="hps", bufs=2, space="PSUM") as hps,
            tc.tile_pool(name="gps", bufs=1, space="PSUM") as gps,
            tc.tile_pool(name="mps", bufs=1, space="PSUM") as mps,
        ):
            # ---- resident loads ----
            def load(t_dram, shape, dtype):
                t = res.tile(shape, dtype, tag=t_dram.name)
                nc.sync.dma_start(t[:], t_dram[:])
                return t

            idx_t = [load(idxlo, [128, max(nch_cls[0] * 8, 8)], i16),
                     load(idxhi, [128, max(nch_cls[1] * 8, 8)], i16)]
            meta_t = load(meta, [128, 2 * ncht], f32)
            w_t = {n: load(w_in[n], [F, F], f16) for n in ("w0", "w1", "w2")}
            wm1_t = load(wm1, [F, NH], f16)
            wm2_t = [None] * 4
            for h in range(4):
                wm2_t[h] = res.tile([128, NO], f16, tag=f"wm2_{h}", name=f"wm2t{h}")
                nc.sync.dma_start(wm2_t[h][:], wm2[128 * h:128 * (h + 1), :])
            b_t = {n: load(b_in[n], [128, F], f32) for n in ("b0r", "b1r", "b2r")}
            bm1c_t = load(bm1c, [128, 4], f32)
            bm2r_t = load(bm2r, [128, NO], f32)
            iota_t = load(iota, [128, G], f16)
            bcol_t = load(bcol, [128, TILES], f32)
            invc_t = load(invc, [128, TILES], f32)

            layer_w = [("w0", "b0r", True), ("w1", "b1r", True), ("w2", "b2r", False)]

            for l in range(3):
                tbl = xtab if l == 0 else tabn
                tbl_ap = [tbl[0:LO, :], tbl[LO:NP, :] if HI > 0 else None]
                wname, bname, relu = layer_w[l]
                issued = [-1, -1]        # last issued batch per class
                cur = [None, None]       # current msg tile per class
                nbat = [-(-nch_cls[0] // BATCH_CH), -(-nch_cls[1] // BATCH_CH)]
                g = 0
                for w in range(NWIN):
                    width = min(WINW, SH - w * WINW)
                    chunks = schedule[w]
                    ps = wps.tile([128, WINW], f32, tag="wps")
                    for j, (cls, cid) in enumerate(chunks):
                        b, slab = divmod(cid, BATCH_CH)
                        if b != issued[cls]:
                            nb = min(BATCH_CH, nch_cls[cls] - b * BATCH_CH)
                            mt = msgp.tile([128, BATCH_CH, F], f16, tag=f"msg{cls}")
                            if variant == "memset":
                                nc.vector.memset(mt[:, :nb, :], 0.0)
                            elif variant not in ("nogather", "nogather_nocc"):
                                nc.gpsimd.dma_gather(
                                    mt[:, :nb, :], tbl_ap[cls],
                                    idx_t[cls][:, b * (BATCH_CH * 8):
                                               b * (BATCH_CH * 8) + nb * 8],
                                    nb * CH, nb * CH, F, single_packet=False)
                            issued[cls] = b
                            cur[cls] = mt
                        if variant in ("gatheronly", "gs", "gsm"):
                            g += 1
                            continue
                        S = sp.tile([128, WINW], f16, tag="S")
                        if variant == "gs":
                            nc.vector.tensor_scalar(
                                out=S[:, :width], in0=iota_t[:, :width],
                                scalar1=meta_t[:, 2 * g:2 * g + 1],
                                scalar2=meta_t[:, 2 * g + 1:2 * g + 2],
                                op0=mybir.AluOpType.is_equal,
                                op1=mybir.AluOpType.mult)
                            g += 1
                            continue
                        nc.vector.tensor_scalar(
                            out=S[:, :width], in0=iota_t[:, :width],
                            scalar1=meta_t[:, 2 * g:2 * g + 1],
                            scalar2=meta_t[:, 2 * g + 1:2 * g + 2],
                            op0=mybir.AluOpType.is_equal,
                            op1=mybir.AluOpType.mult)
                        nc.tensor.matmul(
                            out=ps[:, :width], lhsT=cur[cls][:, slab, :],
                            rhs=S[:, :width],
                            start=(j == 0), stop=(j == len(chunks) - 1))
                        g += 1
                    if variant in ("gatheronly", "gs"):
                        continue
                    aggT = aggp.tile([128, WINW], f16, tag="aggT")
                    if variant == "gsm":
                        nc.vector.tensor_copy(aggT[:, :width], ps[:, :width])
                        continue
                    nc.vector.tensor_copy(aggT[:, :width], ps[:, :width])
                    for sub in range(width // 128):
                        t_idx = w * (WINW // 128) + sub
                        hp = hps.tile([128, F], f32, tag="hp")
                        nc.tensor.matmul(
                            out=hp[:], lhsT=aggT[:, sub * 128:(sub + 1) * 128],
                            rhs=w_t[wname][:], start=True, stop=True)
                        tmp = tmpp.tile([128, F], f32, tag="htmp")
                        nc.vector.tensor_tensor(
                            out=tmp[:], in0=hp[:], in1=b_t[bname][:],
                            op=mybir.AluOpType.add)
                        dst_sl = shb[:, t_idx * F:(t_idx + 1) * F]
                        if relu:
                            nc.vector.tensor_scalar(
                                out=dst_sl, in0=tmp[:], scalar1=0.0, scalar2=None,
                                op0=mybir.AluOpType.max)
                        else:
                            nc.vector.tensor_copy(dst_sl, tmp[:])
                assert g == ncht
                if variant in ("gatheronly", "gs", "gsm"):
                    continue
                if l < 2:
                    nc.sync.dma_start(
                        shard_d.ap().rearrange("(t p) f -> p t f", p=128),
                        shb[:, :].rearrange("p (t f) -> p t f", f=F))
                    if variant not in ("nocc", "nogather_nocc"):
                        nc.gpsimd.collective_compute(
                            "AllGather", mybir.AluOpType.bypass,
                            replica_groups=[list(range(NCORES))],
                            ins=[shard_d[:].opt()], outs=[tabn[:].opt()])

            # ---- mean pool ----
            if variant in ("gatheronly", "gs", "gsm"):
                # touch shb so it exists; write zeros tile to out to keep outputs
                z = tmpp.tile([128, NO], f32, tag="ot", name="zot")
                nc.vector.memset(z[:], 0.0)
                nc.vector.tensor_copy(shb[:, 0:NO], z[:])
                for gh in range(G // 128):
                    nc.sync.dma_start(out[128 * gh:128 * (gh + 1), :], z[:])
                raise _SkipRest
            gp = gps.tile([128, G], f32, tag="gp")
            for t in range(TILES):
                Gt = sp.tile([128, G], f16, tag="S")
                nc.vector.tensor_scalar(
                    out=Gt[:], in0=iota_t[:],
                    scalar1=bcol_t[:, t:t + 1], scalar2=invc_t[:, t:t + 1],
                    op0=mybir.AluOpType.is_equal, op1=mybir.AluOpType.mult)
                nc.tensor.matmul(out=gp[:], lhsT=shb[:, t * F:(t + 1) * F],
                                 rhs=Gt[:], start=(t == 0), stop=(t == TILES - 1))
            gtile = tmpp.tile([128, G], f32, tag="gtile")
            nc.vector.tensor_copy(gtile[:], gp[:])
            nc.sync.dma_start(gt_in[:], gtile[:])
            if variant not in ("nocc", "nogather_nocc"):
                nc.gpsimd.collective_compute(
                    "AllReduce", mybir.AluOpType.add,
                    replica_groups=[list(range(NCORES))],
                    ins=[gt_in[:].opt()], outs=[gt_out[:].opt()])
            gt16 = tmpp.tile([128, G], f16, tag="gt16")
            gfull = tmpp.tile([128, G], f32, tag="gfull")
            nc.sync.dma_start(gfull[:], gt_out[:])
            nc.vector.tensor_copy(gt16[:], gfull[:])

            # ---- MLP ----
            mt16 = []
            for h in range(4):
                mp = mps.tile([128, G], f32, tag="mp")
                nc.tensor.matmul(out=mp[:], lhsT=wm1_t[:, 128 * h:128 * (h + 1)],
                                 rhs=gt16[:], start=True, stop=True)
                mtile = tmpp.tile([128, G], f16, tag=f"mt{h}", name=f"mtile{h}")
                nc.vector.tensor_scalar(
                    out=mtile[:], in0=mp[:], scalar1=bm1c_t[:, h:h + 1],
                    scalar2=0.0, op0=mybir.AluOpType.add, op1=mybir.AluOpType.max)
                mt16.append(mtile)
            for gh in range(G // 128):
                op = mps.tile([128, NO], f32, tag="mp", name="op")
                for h in range(4):
                    nc.tensor.matmul(
                        out=op[:], lhsT=mt16[h][:, 128 * gh:128 * (gh + 1)],
                        rhs=wm2_t[h][:], start=(h == 0), stop=(h == 3))
                ot = tmpp.tile([128, NO], f32, tag="ot")
                nc.vector.tensor_tensor(out=ot[:], in0=op[:], in1=bm2r_t[:],
                                        op=mybir.AluOpType.add)
                nc.sync.dma_start(out[128 * gh:128 * (gh + 1), :], ot[:])

    nc.compile()
    return nc


def _get_built(inputs):
    import hashlib
    h = hashlib.sha1()
    h.update(np.ascontiguousarray(inputs["edge_index"]).tobytes())
    h.update(np.ascontiguousarray(inputs["batch"]).tobytes())
    key = (tuple(sorted((k, v.shape, str(v.dtype)) for k, v in inputs.items())),
           h.hexdigest())
    if key not in _cache:
        geom, in_maps = _host_prep(**inputs)
        nc = _build_bass(geom)
        _cache[key] = (geom, nc)
    else:
        geom, nc = _cache[key]
        _, in_maps = _host_prep(**inputs)
    return geom, nc, in_maps


def kernel(**inputs):
    inputs = {k: np.asarray(v) for k, v in inputs.items()}
    geom, nc, in_maps = _get_built(inputs)
    from concourse.bass_utils import run_bass_kernel_spmd
    res = run_bass_kernel_spmd(nc, in_maps, list(range(NCORES)))
    return np.asarray(res.results[0]["out"])



# revision 8
# speedup vs baseline: 1.7787x; 1.7787x over previous
"""GCN encoder (3x GCNConv + mean-pool + MLP) as an 8-core Trainium2 Bass kernel.

Sharding: nodes/edges partitioned by destination-node owner (8 shards).
All per-core constant data (x shard, edge-index streams, per-chunk metadata,
weights, pooling helpers) is packed host-side into ONE int16 DRAM blob per
core — per-execution overhead in this environment scales with the number of
bound tensors, so the kernel binds exactly one input and one output.

Per execution: the x shard is staged to DRAM and AllGather'd into a
replicated [NP, F] f16 feature table.  Per layer: per-edge source features
are fetched with dma_gather, scaled+scattered into per-destination-window
sums via a PE matmul against a one-hot selection matrix built on DVE, then
the layer weight matmul + bias + ReLU produces this core's shard of the
next layer's features, which an AllGather replicates into every core's
table.  Mean-pool is a matmul against a per-graph one-hot (scaled by
1/count), AllReduce-summed across cores; the tiny MLP runs replicated.
"""

import numpy as np

NCORES = 8
F = 128            # hidden width (all layers padded to this)
G = 256            # number of graphs
NH = 512           # MLP hidden
NO = 256           # MLP out
CH = 128           # edges per chunk
BATCH_CH = 32      # chunks per dma_gather batch
WINW = 256         # dst nodes per PSUM accumulation window

_cache = {}


def _host_prep(x, edge_index, batch, W0, b0, W1, b1, W2, b2, Wm1, bm1, Wm2, bm2):
    N = x.shape[0]
    FI = x.shape[1]
    SH = -(-N // (NCORES * 128)) * 128      # shard size (nodes), 128-multiple
    NP = SH * NCORES
    TILES = SH // 128
    NWIN = -(-SH // WINW)
    LO = min(32768, NP)
    HI = NP - LO

    src = np.concatenate([edge_index[0], np.arange(N, dtype=np.int64)])
    dst = np.concatenate([edge_index[1], np.arange(N, dtype=np.int64)])
    deg = np.bincount(dst, minlength=N).astype(np.float32)
    dis = np.where(deg > 0, 1.0 / np.sqrt(np.maximum(deg, 1.0)), 0.0).astype(np.float32)
    norm = dis[src] * dis[dst]

    xpad = np.zeros((NP, F), dtype=np.float16)
    xpad[:N, :FI] = x.astype(np.float16)

    # per-core edge selection, ordered by (window, class, dst)
    per_core = []
    for c in range(NCORES):
        base = c * SH
        sel = (dst >= base) & (dst < base + SH)
        es = src[sel].astype(np.int64)
        ed = (dst[sel] - base).astype(np.int64)
        en = norm[sel]
        cl = (es >= LO).astype(np.int64)
        wi = ed // WINW
        order = np.lexsort((ed, cl, wi))
        per_core.append((es[order], ed[order], en[order], cl[order], wi[order]))

    # chunk counts per (window, class), equalized across cores
    counts = np.zeros((NCORES, NWIN, 2), dtype=np.int64)
    for c in range(NCORES):
        _, _, _, cl, wi = per_core[c]
        for cls in (0, 1):
            cnt = np.bincount(wi[cl == cls], minlength=NWIN)
            counts[c, :, cls] = cnt
    nch = -(-counts.max(axis=0) // CH)  # [NWIN, 2] chunks
    nch_cls = nch.sum(axis=0)          # total chunks per class
    ncht = int(nch.sum())

    # shared program schedule: windows -> list of (cls, cid); meta col = global g
    schedule = []
    cid_ctr = [0, 0]
    for w in range(NWIN):
        lst = []
        for cls in (0, 1):
            for _ in range(int(nch[w, cls])):
                lst.append((cls, cid_ctr[cls]))
                cid_ctr[cls] += 1
        schedule.append(lst)

    # per-core streams
    idx_streams = [[], []]   # per class: list over cores of [16, n/16] int16
    metas = []
    for c in range(NCORES):
        es, ed, en, cl, wi = per_core[c]
        idx_parts = [[], []]
        meta = np.zeros((128, 2 * ncht), dtype=np.float32)
        g = 0
        pos = 0
        # edges are sorted (win, cls, dst); walk groups in the same order
        for w in range(NWIN):
            for cls in (0, 1):
                n_e = int(counts[c, w, cls])
                tot = int(nch[w, cls]) * CH
                ge, gd, gn = es[pos:pos + n_e], ed[pos:pos + n_e], en[pos:pos + n_e]
                pos += n_e
                pad = tot - n_e
                iv = ge - (LO if cls else 0)
                iv = np.concatenate([iv, np.zeros(pad, np.int64)])
                dl = np.concatenate([gd - w * WINW, np.zeros(pad, np.int64)])
                nr = np.concatenate([gn, np.zeros(pad, np.float32)])
                idx_parts[cls].append(iv.astype(np.int16))
                for k in range(tot // CH):
                    meta[:, 2 * g] = dl[k * CH:(k + 1) * CH].astype(np.float32)
                    meta[:, 2 * g + 1] = nr[k * CH:(k + 1) * CH].astype(np.float32)
                    g += 1
        assert g == ncht
        for cls in (0, 1):
            arr = (np.concatenate(idx_parts[cls]) if idx_parts[cls]
                   else np.zeros(0, np.int16))
            assert arr.size == nch_cls[cls] * CH
            if arr.size:
                wrapped = np.ascontiguousarray(arr.reshape(-1, 16).T)  # [16, n/16]
            else:
                wrapped = np.zeros((16, 8), np.int16)  # dummy
            idx_streams[cls].append(wrapped)
        metas.append(meta)

    # pooling helpers
    cnt = np.bincount(batch.astype(np.int64), minlength=G).astype(np.float32)
    invc_all = (1.0 / np.maximum(cnt, 1.0))[batch.astype(np.int64)]
    bcols, invcs = [], []
    for c in range(NCORES):
        sl = slice(c * SH, min((c + 1) * SH, N))
        b_sh = np.zeros(SH, np.float32)
        i_sh = np.zeros(SH, np.float32)
        nreal = max(0, min((c + 1) * SH, N) - c * SH)
        if nreal > 0:
            b_sh[:nreal] = batch[sl].astype(np.float32)
            i_sh[:nreal] = invc_all[sl].astype(np.float32)
        bcols.append(np.ascontiguousarray(b_sh.reshape(TILES, 128).T))  # [128,TILES]
        invcs.append(np.ascontiguousarray(i_sh.reshape(TILES, 128).T))

    W0p = np.zeros((F, F), np.float16)
    W0p[:FI] = W0.astype(np.float16)

    # ---- blob packing ----
    # Section table: name -> (shape, dtype).  All cores share shapes; offsets
    # are in int16 elements, 128-aligned.
    n0 = max(int(nch_cls[0]) * 8, 8)
    n1 = max(int(nch_cls[1]) * 8, 8)
    sections = [
        ("xsh", (128, TILES * F), np.float16),
        ("idxlo", (16, n0), np.int16),
        ("idxhi", (16, n1), np.int16),
        ("meta", (128, 2 * ncht), np.float32),
        ("w0", (F, F), np.float16),
        ("w1", (F, F), np.float16),
        ("w2", (F, F), np.float16),
        ("wm1", (F, NH), np.float16),
        ("wm2", (NH, NO), np.float16),
        ("iota", (128, G), np.float16),
        ("b0r", (128, F), np.float32),
        ("b1r", (128, F), np.float32),
        ("b2r", (128, F), np.float32),
        ("bm1c", (128, 4), np.float32),
        ("bm2r", (128, NO), np.float32),
        ("bcol", (128, TILES), np.float32),
        ("invc", (128, TILES), np.float32),
    ]
    offs = {}
    pos16 = 0
    for name, shape, dtype in sections:
        n16 = int(np.prod(shape)) * np.dtype(dtype).itemsize // 2
        offs[name] = (pos16, shape, dtype)
        pos16 += -(-n16 // 128) * 128
    TOT = pos16

    consts = {
        "w0": W0p, "w1": W1.astype(np.float16), "w2": W2.astype(np.float16),
        "wm1": Wm1.astype(np.float16), "wm2": Wm2.astype(np.float16),
        "b0r": np.tile(b0.astype(np.float32)[None, :], (128, 1)),
        "b1r": np.tile(b1.astype(np.float32)[None, :], (128, 1)),
        "b2r": np.tile(b2.astype(np.float32)[None, :], (128, 1)),
        "bm1c": np.ascontiguousarray(bm1.astype(np.float32).reshape(4, 128).T),
        "bm2r": np.tile(bm2.astype(np.float32)[None, :], (128, 1)),
        "iota": np.tile(np.arange(G, dtype=np.float16)[None, :], (128, 1)),
    }
    in_maps = []
    for c in range(NCORES):
        blob = np.zeros(TOT, np.int16)

        def put(name, arr):
            off, shape, dtype = offs[name]
            a = np.ascontiguousarray(arr.astype(dtype, copy=False))
            assert a.shape == shape, (name, a.shape, shape)
            v = a.view(np.int16).reshape(-1)
            blob[off:off + v.size] = v

        xstage = (xpad[c * SH:(c + 1) * SH]
                  .reshape(TILES, 128, F).transpose(1, 0, 2).reshape(128, TILES * F))
        put("xsh", xstage)
        ilo = idx_streams[0][c]
        ihi = idx_streams[1][c]
        ilo_p = np.zeros((16, n0), np.int16); ilo_p[:, :ilo.shape[1]] = ilo
        ihi_p = np.zeros((16, n1), np.int16); ihi_p[:, :ihi.shape[1]] = ihi
        put("idxlo", ilo_p)
        put("idxhi", ihi_p)
        put("meta", metas[c])
        for k, v in consts.items():
            put(k, v)
        put("bcol", bcols[c])
        put("invc", invcs[c])
        in_maps.append({"blob": blob.reshape(1, TOT)})

    geom = dict(N=N, NP=NP, SH=SH, TILES=TILES, NWIN=NWIN, LO=LO, HI=HI,
                nch=nch, nch_cls=[int(v) for v in nch_cls], ncht=ncht,
                schedule=schedule, offs=offs, TOT=TOT, n0=n0, n1=n1)
    return geom, in_maps


class _SkipRest(Exception):
    pass


def _build_bass(geom, variant="full", repeat=1):
    import concourse.bass as bass
    import concourse.tile as tile
    from concourse import bacc, mybir

    f16, f32, i16 = mybir.dt.float16, mybir.dt.float32, mybir.dt.int16
    dt_map = {np.float16: f16, np.float32: f32, np.int16: i16}
    NP, SH, TILES, NWIN = geom["NP"], geom["SH"], geom["TILES"], geom["NWIN"]
    LO, HI = geom["LO"], geom["HI"]
    nch, nch_cls, ncht = geom["nch"], geom["nch_cls"], geom["ncht"]
    schedule, offs, TOT = geom["schedule"], geom["offs"], geom["TOT"]
    n0, n1 = geom["n0"], geom["n1"]

    nc = bacc.Bacc("TRN2", target_bir_lowering=False, debug=False,
                   num_devices=NCORES)

    blob = nc.dram_tensor("blob", [1, TOT], i16, kind="ExternalInput")
    out = nc.dram_tensor("out", [G, NO], f32, kind="ExternalOutput")

    shard_d = nc.dram_tensor("shard_d", [SH, F], f16)
    tabn = nc.dram_tensor("tabn", [NP, F], f16, addr_space="Shared")
    gt_in = nc.dram_tensor("gt_in", [128, G], f32)
    gt_out = nc.dram_tensor("gt_out", [128, G], f32, addr_space="Shared")

    shb = nc.alloc_sbuf_tensor("shb", [128, TILES * F], f16)

    def sec(name):
        off, shape, dtype = offs[name]
        nel = int(np.prod(shape))
        bdt = dt_map[dtype]
        n16 = nel * np.dtype(dtype).itemsize // 2
        ap = blob[0, off:off + n16].bitcast(bdt)
        return ap.rearrange("(p n) -> p n", p=shape[0])

    import contextlib
    with tile.TileContext(nc) as tc:
        with (
            contextlib.suppress(_SkipRest),
            tc.tile_pool(name="res", bufs=1) as res,
            tc.tile_pool(name="msg", bufs=3) as msgp,
            tc.tile_pool(name="sp", bufs=4) as sp,
            tc.tile_pool(name="agg", bufs=2) as aggp,
            tc.tile_pool(name="tmp", bufs=2) as tmpp,
            tc.tile_pool(name="wps", bufs=2, space="PSUM") as wps,
            tc.tile_pool(name="hps", bufs=2, space="PSUM") as hps,
            tc.tile_pool(name="gps", bufs=1, space="PSUM") as gps,
            tc.tile_pool(name="mps", bufs=1, space="PSUM") as mps,
        ):
          for rep in range(repeat):
            # ---- resident loads (all from the blob) ----
            def load(name, shape, dtype):
                t = res.tile(list(shape), dtype, tag=name)
                nc.sync.dma_start(t[:], sec(name))
                return t

            idx_t = []
            for cls, nn in ((0, n0), (1, n1)):
                t = res.tile([128, nn], i16, tag=f"idx{cls}")
                src_ap = sec(f"idx{'lo' if cls == 0 else 'hi'}")
                for k in range(8):
                    nc.sync.dma_start(t[16 * k:16 * (k + 1), :], src_ap)
                idx_t.append(t)
            meta_t = load("meta", [128, 2 * ncht], f32)
            w_t = {n: load(n, [F, F], f16) for n in ("w0", "w1", "w2")}
            wm1_t = load("wm1", [F, NH], f16)
            wm2_sec = sec("wm2")
            wm2_t = [None] * 4
            for h in range(4):
                wm2_t[h] = res.tile([128, NO], f16, tag=f"wm2_{h}", name=f"wm2t{h}")
                nc.sync.dma_start(wm2_t[h][:], wm2_sec[128 * h:128 * (h + 1), :])
            b_t = {n: load(n, [128, F], f32) for n in ("b0r", "b1r", "b2r")}
            bm1c_t = load("bm1c", [128, 4], f32)
            bm2r_t = load("bm2r", [128, NO], f32)
            iota_t = load("iota", [128, G], f16)
            bcol_t = load("bcol", [128, TILES], f32)
            invc_t = load("invc", [128, TILES], f32)

            # ---- stage x shard -> AllGather into the shared feature table ----
            xstage = res.tile([128, TILES * F], f16, tag="xstage")
            nc.sync.dma_start(xstage[:], sec("xsh"))
            nc.sync.dma_start(
                shard_d.ap().rearrange("(t p) f -> p t f", p=128),
                xstage[:].rearrange("p (t f) -> p t f", f=F))
            if variant not in ("nocc", "nogather_nocc"):
                nc.gpsimd.collective_compute(
                    "AllGather", mybir.AluOpType.bypass,
                    replica_groups=[list(range(NCORES))],
                    ins=[shard_d[:].opt()], outs=[tabn[:].opt()])

            layer_w = [("w0", "b0r", True), ("w1", "b1r", True), ("w2", "b2r", False)]

            if variant == "cconly":
                nc.vector.memset(shb[:, :], 0.0)
            for l in range(3):
                if variant == "cconly":
                    if l < 2:
                        nc.sync.dma_start(
                            shard_d.ap().rearrange("(t p) f -> p t f", p=128),
                            shb[:, :].rearrange("p (t f) -> p t f", f=F))
                        nc.gpsimd.collective_compute(
                            "AllGather", mybir.AluOpType.bypass,
                            replica_groups=[list(range(NCORES))],
                            ins=[shard_d[:].opt()], outs=[tabn[:].opt()])
                    continue
                tbl_ap = [tabn[0:LO, :], tabn[LO:NP, :] if HI > 0 else None]
                wname, bname, relu = layer_w[l]
                issued = [-1, -1]        # last issued batch per class
                cur = [None, None]       # current msg tile per class
                g = 0
                for w in range(NWIN):
                    width = min(WINW, SH - w * WINW)
                    chunks = schedule[w]
                    if variant not in ("gatheronly", "gs"):
                        ps = wps.tile([128, WINW], f32, tag="wps", name="ps")
                    else:
                        ps = None
                    for j, (cls, cid) in enumerate(chunks):
                        b, slab = divmod(cid, BATCH_CH)
                        if b != issued[cls]:
                            nb = min(BATCH_CH, nch_cls[cls] - b * BATCH_CH)
                            mt = msgp.tile([128, BATCH_CH, F], f16, tag=f"msg{cls}")
                            if variant == "memset":
                                nc.vector.memset(mt[:, :nb, :], 0.0)
                            elif variant not in ("nogather", "nogather_nocc"):
                                nc.gpsimd.dma_gather(
                                    mt[:, :nb, :], tbl_ap[cls],
                                    idx_t[cls][:, b * (BATCH_CH * 8):
                                               b * (BATCH_CH * 8) + nb * 8],
                                    nb * CH, nb * CH, F, single_packet=False)
                            issued[cls] = b
                            cur[cls] = mt
                        if variant in ("gatheronly",):
                            g += 1
                            continue
                        S = sp.tile([128, WINW], f16, tag="S")
                        nc.vector.tensor_scalar(
                            out=S[:, :width], in0=iota_t[:, :width],
                            scalar1=meta_t[:, 2 * g:2 * g + 1],
                            scalar2=meta_t[:, 2 * g + 1:2 * g + 2],
                            op0=mybir.AluOpType.is_equal,
                            op1=mybir.AluOpType.mult)
                        if variant == "gs":
                            g += 1
                            continue
                        nc.tensor.matmul(
                            out=ps[:, :width], lhsT=cur[cls][:, slab, :],
                            rhs=S[:, :width],
                            start=(j == 0), stop=(j == len(chunks) - 1))
                        g += 1
                    if variant in ("gatheronly", "gs"):
                        continue
                    aggT = aggp.tile([128, WINW], f16, tag="aggT")
                    nc.vector.tensor_copy(aggT[:, :width], ps[:, :width])
                    if variant == "gsm":
                        continue
                    for sub in range(width // 128):
                        t_idx = w * (WINW // 128) + sub
                        hp = hps.tile([128, F], f32, tag="hp")
                        nc.tensor.matmul(
                            out=hp[:], lhsT=aggT[:, sub * 128:(sub + 1) * 128],
                            rhs=w_t[wname][:], start=True, stop=True)
                        tmp = tmpp.tile([128, F], f32, tag="htmp")
                        nc.vector.tensor_tensor(
                            out=tmp[:], in0=hp[:], in1=b_t[bname][:],
                            op=mybir.AluOpType.add)
                        dst_sl = shb[:, t_idx * F:(t_idx + 1) * F]
                        if relu:
                            nc.vector.tensor_scalar(
                                out=dst_sl, in0=tmp[:], scalar1=0.0, scalar2=None,
                                op0=mybir.AluOpType.max)
                        else:
                            nc.vector.tensor_copy(dst_sl, tmp[:])
                if variant in ("gatheronly", "gs", "gsm"):
                    continue
                assert g == ncht
                if l < 2:
                    nc.sync.dma_start(
                        shard_d.ap().rearrange("(t p) f -> p t f", p=128),
                        shb[:, :].rearrange("p (t f) -> p t f", f=F))
                    if variant not in ("nocc", "nogather_nocc"):
                        nc.gpsimd.collective_compute(
                            "AllGather", mybir.AluOpType.bypass,
                            replica_groups=[list(range(NCORES))],
                            ins=[shard_d[:].opt()], outs=[tabn[:].opt()])

            # ---- mean pool ----
            if variant in ("gatheronly", "gs", "gsm"):
                if rep < repeat - 1:
                    continue
                z = tmpp.tile([128, NO], f32, tag="ot", name="zot")
                nc.vector.memset(z[:], 0.0)
                nc.vector.tensor_copy(shb[:, 0:NO], z[:])
                for gh in range(G // 128):
                    nc.sync.dma_start(out[128 * gh:128 * (gh + 1), :], z[:])
                raise _SkipRest
            gp = gps.tile([128, G], f32, tag="gp")
            for t in range(TILES):
                Gt = sp.tile([128, G], f16, tag="S")
                nc.vector.tensor_scalar(
                    out=Gt[:], in0=iota_t[:],
                    scalar1=bcol_t[:, t:t + 1], scalar2=invc_t[:, t:t + 1],
                    op0=mybir.AluOpType.is_equal, op1=mybir.AluOpType.mult)
                nc.tensor.matmul(out=gp[:], lhsT=shb[:, t * F:(t + 1) * F],
                                 rhs=Gt[:], start=(t == 0), stop=(t == TILES - 1))
            gtile = tmpp.tile([128, G], f32, tag="gtile")
            nc.vector.tensor_copy(gtile[:], gp[:])
            nc.sync.dma_start(gt_in[:], gtile[:])
            if variant not in ("nocc", "nogather_nocc"):
                nc.gpsimd.collective_compute(
                    "AllReduce", mybir.AluOpType.add,
                    replica_groups=[list(range(NCORES))],
                    ins=[gt_in[:].opt()], outs=[gt_out[:].opt()])
            gt16 = tmpp.tile([128, G], f16, tag="gt16")
            gfull = tmpp.tile([128, G], f32, tag="gfull")
            nc.sync.dma_start(gfull[:], gt_out[:])
            nc.vector.tensor_copy(gt16[:], gfull[:])

            # ---- MLP ----
            mt16 = []
            for h in range(4):
                mp = mps.tile([128, G], f32, tag="mp")
                nc.tensor.matmul(out=mp[:], lhsT=wm1_t[:, 128 * h:128 * (h + 1)],
                                 rhs=gt16[:], start=True, stop=True)
                mtile = tmpp.tile([128, G], f16, tag=f"mt{h}", name=f"mtile{h}")
                nc.vector.tensor_scalar(
                    out=mtile[:], in0=mp[:], scalar1=bm1c_t[:, h:h + 1],
                    scalar2=0.0, op0=mybir.AluOpType.add, op1=mybir.AluOpType.max)
                mt16.append(mtile)
            for gh in range(G // 128):
                op = mps.tile([128, NO], f32, tag="mp", name="op")
                for h in range(4):
                    nc.tensor.matmul(
                        out=op[:], lhsT=mt16[h][:, 128 * gh:128 * (gh + 1)],
                        rhs=wm2_t[h][:], start=(h == 0), stop=(h == 3))
                ot = tmpp.tile([128, NO], f32, tag="ot")
                nc.vector.tensor_tensor(out=ot[:], in0=op[:], in1=bm2r_t[:],
                                        op=mybir.AluOpType.add)
                nc.sync.dma_start(out[128 * gh:128 * (gh + 1), :], ot[:])

    nc.compile()
    return nc


def _get_built(inputs):
    import hashlib
    h = hashlib.sha1()
    for k in sorted(inputs):
        h.update(k.encode())
        h.update(np.ascontiguousarray(inputs[k]).tobytes())
    key = h.hexdigest()
    if key not in _cache:
        geom, in_maps = _host_prep(**inputs)
        nc = _build_bass(geom)
        _cache[key] = (geom, nc, in_maps)
    return _cache[key]


def kernel(**inputs):
    inputs = {k: np.asarray(v) for k, v in inputs.items()}
    geom, nc, in_maps = _get_built(inputs)
    from concourse.bass_utils import run_bass_kernel_spmd
    res = run_bass_kernel_spmd(nc, in_maps, list(range(NCORES)))
    return np.asarray(res.results[0]["out"])


# revision 10
# speedup vs baseline: 9.1368x; 5.1368x over previous
"""GCN encoder (3x GCNConv + mean-pool + MLP) as an 8-core Trainium2 Bass kernel.

Sharding: nodes/edges partitioned by destination-node owner (8 shards).
All per-core constant data (x shard, edge-index streams, per-chunk metadata,
weights, pooling helpers) is packed host-side into ONE int16 DRAM blob per
core — per-execution overhead in this environment scales with the number of
bound tensors, so the kernel binds exactly one input and one output.

Per execution: the x shard is staged to DRAM and AllGather'd into a
replicated [NP, F] f16 feature table.  Per layer: per-edge source features
are fetched with dma_gather, scaled+scattered into per-destination-window
sums via a PE matmul against a one-hot selection matrix built on DVE, then
the layer weight matmul + bias + ReLU produces this core's shard of the
next layer's features, which an AllGather replicates into every core's
table.  Mean-pool is a matmul against a per-graph one-hot (scaled by
1/count), AllReduce-summed across cores; the tiny MLP runs replicated.
"""

import numpy as np

NCORES = 8
F = 128            # hidden width (all layers padded to this)
G = 256            # number of graphs
NH = 512           # MLP hidden
NO = 256           # MLP out
CH = 128           # edges per chunk
BATCH_CH = 32      # chunks per dma_gather batch
WINW = 256         # dst nodes per PSUM accumulation window

_cache = {}


def _host_prep(x, edge_index, batch, W0, b0, W1, b1, W2, b2, Wm1, bm1, Wm2, bm2):
    N = x.shape[0]
    FI = x.shape[1]
    SH = -(-N // (NCORES * 128)) * 128      # shard size (nodes), 128-multiple
    NP = SH * NCORES
    TILES = SH // 128
    NWIN = -(-SH // WINW)
    LO = min(32768, NP)
    HI = NP - LO

    src = np.concatenate([edge_index[0], np.arange(N, dtype=np.int64)])
    dst = np.concatenate([edge_index[1], np.arange(N, dtype=np.int64)])
    deg = np.bincount(dst, minlength=N).astype(np.float32)
    dis = np.where(deg > 0, 1.0 / np.sqrt(np.maximum(deg, 1.0)), 0.0).astype(np.float32)
    norm = dis[src] * dis[dst]

    xpad = np.zeros((NP, F), dtype=np.float16)
    xpad[:N, :FI] = x.astype(np.float16)

    # per-core edge selection, ordered by (window, class, dst)
    per_core = []
    for c in range(NCORES):
        base = c * SH
        sel = (dst >= base) & (dst < base + SH)
        es = src[sel].astype(np.int64)
        ed = (dst[sel] - base).astype(np.int64)
        en = norm[sel]
        cl = (es >= LO).astype(np.int64)
        wi = ed // WINW
        order = np.lexsort((ed, cl, wi))
        per_core.append((es[order], ed[order], en[order], cl[order], wi[order]))

    # chunk counts per (window, class), equalized across cores
    counts = np.zeros((NCORES, NWIN, 2), dtype=np.int64)
    for c in range(NCORES):
        _, _, _, cl, wi = per_core[c]
        for cls in (0, 1):
            cnt = np.bincount(wi[cl == cls], minlength=NWIN)
            counts[c, :, cls] = cnt
    nch = -(-counts.max(axis=0) // CH)  # [NWIN, 2] chunks
    nch_cls = nch.sum(axis=0)          # total chunks per class
    ncht = int(nch.sum())

    # shared program schedule: windows -> list of (cls, cid); meta col = global g
    schedule = []
    cid_ctr = [0, 0]
    for w in range(NWIN):
        lst = []
        for cls in (0, 1):
            for _ in range(int(nch[w, cls])):
                lst.append((cls, cid_ctr[cls]))
                cid_ctr[cls] += 1
        schedule.append(lst)

    # per-core streams
    idx_streams = [[], []]   # per class: list over cores of [16, n/16] int16
    metas = []
    for c in range(NCORES):
        es, ed, en, cl, wi = per_core[c]
        idx_parts = [[], []]
        meta = np.zeros((128, 2 * ncht), dtype=np.float32)
        g = 0
        pos = 0
        # edges are sorted (win, cls, dst); walk groups in the same order
        for w in range(NWIN):
            for cls in (0, 1):
                n_e = int(counts[c, w, cls])
                tot = int(nch[w, cls]) * CH
                ge, gd, gn = es[pos:pos + n_e], ed[pos:pos + n_e], en[pos:pos + n_e]
                pos += n_e
                pad = tot - n_e
                iv = ge - (LO if cls else 0)
                iv = np.concatenate([iv, np.zeros(pad, np.int64)])
                dl = np.concatenate([gd - w * WINW, np.zeros(pad, np.int64)])
                nr = np.concatenate([gn, np.zeros(pad, np.float32)])
                idx_parts[cls].append(iv.astype(np.int16))
                for k in range(tot // CH):
                    meta[:, 2 * g] = dl[k * CH:(k + 1) * CH].astype(np.float32)
                    meta[:, 2 * g + 1] = nr[k * CH:(k + 1) * CH].astype(np.float32)
                    g += 1
        assert g == ncht
        for cls in (0, 1):
            arr = (np.concatenate(idx_parts[cls]) if idx_parts[cls]
                   else np.zeros(0, np.int16))
            assert arr.size == nch_cls[cls] * CH
            if arr.size:
                wrapped = np.ascontiguousarray(arr.reshape(-1, 16).T)  # [16, n/16]
            else:
                wrapped = np.zeros((16, 8), np.int16)  # dummy
            idx_streams[cls].append(wrapped)
        metas.append(meta)

    # pooling helpers
    cnt = np.bincount(batch.astype(np.int64), minlength=G).astype(np.float32)
    invc_all = (1.0 / np.maximum(cnt, 1.0))[batch.astype(np.int64)]
    bcols, invcs = [], []
    for c in range(NCORES):
        sl = slice(c * SH, min((c + 1) * SH, N))
        b_sh = np.zeros(SH, np.float32)
        i_sh = np.zeros(SH, np.float32)
        nreal = max(0, min((c + 1) * SH, N) - c * SH)
        if nreal > 0:
            b_sh[:nreal] = batch[sl].astype(np.float32)
            i_sh[:nreal] = invc_all[sl].astype(np.float32)
        bcols.append(np.ascontiguousarray(b_sh.reshape(TILES, 128).T))  # [128,TILES]
        invcs.append(np.ascontiguousarray(i_sh.reshape(TILES, 128).T))

    W0p = np.zeros((F, F), np.float16)
    W0p[:FI] = W0.astype(np.float16)

    # ---- blob packing ----
    # Section table: name -> (shape, dtype).  All cores share shapes; offsets
    # are in int16 elements, 128-aligned.
    n0 = max(int(nch_cls[0]) * 8, 8)
    n1 = max(int(nch_cls[1]) * 8, 8)
    sections = [
        ("xsh", (128, TILES * F), np.float16),
        ("idxlo", (16, n0), np.int16),
        ("idxhi", (16, n1), np.int16),
        ("meta", (128, 2 * ncht), np.float32),
        ("w0", (F, F), np.float16),
        ("w1", (F, F), np.float16),
        ("w2", (F, F), np.float16),
        ("wm1", (F, NH), np.float16),
        ("wm2", (NH, NO), np.float16),
        ("iota", (128, G), np.float16),
        ("b0r", (128, F), np.float32),
        ("b1r", (128, F), np.float32),
        ("b2r", (128, F), np.float32),
        ("bm1c", (128, 4), np.float32),
        ("bm2r", (128, NO), np.float32),
        ("bcol", (128, TILES), np.float32),
        ("invc", (128, TILES), np.float32),
    ]
    offs = {}
    pos16 = 0
    for name, shape, dtype in sections:
        n16 = int(np.prod(shape)) * np.dtype(dtype).itemsize // 2
        offs[name] = (pos16, shape, dtype)
        pos16 += -(-n16 // 128) * 128
    TOT = pos16

    consts = {
        "w0": W0p, "w1": W1.astype(np.float16), "w2": W2.astype(np.float16),
        "wm1": Wm1.astype(np.float16), "wm2": Wm2.astype(np.float16),
        "b0r": np.tile(b0.astype(np.float32)[None, :], (128, 1)),
        "b1r": np.tile(b1.astype(np.float32)[None, :], (128, 1)),
        "b2r": np.tile(b2.astype(np.float32)[None, :], (128, 1)),
        "bm1c": np.ascontiguousarray(bm1.astype(np.float32).reshape(4, 128).T),
        "bm2r": np.tile(bm2.astype(np.float32)[None, :], (128, 1)),
        "iota": np.tile(np.arange(G, dtype=np.float16)[None, :], (128, 1)),
    }
    in_maps = []
    for c in range(NCORES):
        blob = np.zeros(TOT, np.int16)

        def put(name, arr):
            off, shape, dtype = offs[name]
            a = np.ascontiguousarray(arr.astype(dtype, copy=False))
            assert a.shape == shape, (name, a.shape, shape)
            v = a.view(np.int16).reshape(-1)
            blob[off:off + v.size] = v

        xstage = (xpad[c * SH:(c + 1) * SH]
                  .reshape(TILES, 128, F).transpose(1, 0, 2).reshape(128, TILES * F))
        put("xsh", xstage)
        ilo = idx_streams[0][c]
        ihi = idx_streams[1][c]
        ilo_p = np.zeros((16, n0), np.int16); ilo_p[:, :ilo.shape[1]] = ilo
        ihi_p = np.zeros((16, n1), np.int16); ihi_p[:, :ihi.shape[1]] = ihi
        put("idxlo", ilo_p)
        put("idxhi", ihi_p)
        put("meta", metas[c])
        for k, v in consts.items():
            put(k, v)
        put("bcol", bcols[c])
        put("invc", invcs[c])
        in_maps.append({"blob": blob.reshape(1, TOT)})

    geom = dict(N=N, NP=NP, SH=SH, TILES=TILES, NWIN=NWIN, LO=LO, HI=HI,
                nch=nch, nch_cls=[int(v) for v in nch_cls], ncht=ncht,
                schedule=schedule, offs=offs, TOT=TOT, n0=n0, n1=n1)
    return geom, in_maps


class _SkipRest(Exception):
    pass


def _build_bass(geom, variant="full", repeat=1):
    import concourse.bass as bass
    import concourse.tile as tile
    from concourse import bacc, mybir

    f16, f32, i16 = mybir.dt.float16, mybir.dt.float32, mybir.dt.int16
    dt_map = {np.float16: f16, np.float32: f32, np.int16: i16}
    NP, SH, TILES, NWIN = geom["NP"], geom["SH"], geom["TILES"], geom["NWIN"]
    LO, HI = geom["LO"], geom["HI"]
    nch, nch_cls, ncht = geom["nch"], geom["nch_cls"], geom["ncht"]
    schedule, offs, TOT = geom["schedule"], geom["offs"], geom["TOT"]
    n0, n1 = geom["n0"], geom["n1"]

    nc = bacc.Bacc("TRN2", target_bir_lowering=False, debug=False,
                   num_devices=NCORES)

    blob = nc.dram_tensor("blob", [1, TOT], i16, kind="ExternalInput")
    out = nc.dram_tensor("out", [G, NO], f32, kind="ExternalOutput")

    shard_d = nc.dram_tensor("shard_d", [SH, F], f16)
    tabn = nc.dram_tensor("tabn", [NP, F], f16, addr_space="Shared")
    gt_in = nc.dram_tensor("gt_in", [128, G], f32)
    gt_out = nc.dram_tensor("gt_out", [128, G], f32, addr_space="Shared")

    shb = nc.alloc_sbuf_tensor("shb", [128, TILES * F], f16)

    def sec(name):
        off, shape, dtype = offs[name]
        nel = int(np.prod(shape))
        bdt = dt_map[dtype]
        n16 = nel * np.dtype(dtype).itemsize // 2
        ap = blob[0, off:off + n16].bitcast(bdt)
        return ap.rearrange("(p n) -> p n", p=shape[0])

    import contextlib
    with tile.TileContext(nc) as tc:
        with (
            contextlib.suppress(_SkipRest),
            tc.tile_pool(name="res", bufs=1) as res,
            tc.tile_pool(name="msg", bufs=3) as msgp,
            tc.tile_pool(name="sp", bufs=4) as sp,
            tc.tile_pool(name="agg", bufs=2) as aggp,
            tc.tile_pool(name="tmp", bufs=2) as tmpp,
            tc.tile_pool(name="wps", bufs=2, space="PSUM") as wps,
            tc.tile_pool(name="hps", bufs=2, space="PSUM") as hps,
            tc.tile_pool(name="gps", bufs=1, space="PSUM") as gps,
            tc.tile_pool(name="mps", bufs=1, space="PSUM") as mps,
        ):
          for rep in range(repeat):
            # ---- resident loads (all from the blob) ----
            def load(name, shape, dtype):
                t = res.tile(list(shape), dtype, tag=name)
                nc.sync.dma_start(t[:], sec(name))
                return t

            idx_t = []
            for cls, nn in ((0, n0), (1, n1)):
                t = res.tile([128, nn], i16, tag=f"idx{cls}")
                src_ap = sec(f"idx{'lo' if cls == 0 else 'hi'}")
                for k in range(8):
                    nc.sync.dma_start(t[16 * k:16 * (k + 1), :], src_ap)
                idx_t.append(t)
            meta_t = load("meta", [128, 2 * ncht], f32)
            w_t = {n: load(n, [F, F], f16) for n in ("w0", "w1", "w2")}
            wm1_t = load("wm1", [F, NH], f16)
            wm2_sec = sec("wm2")
            wm2_t = [None] * 4
            for h in range(4):
                wm2_t[h] = res.tile([128, NO], f16, tag=f"wm2_{h}", name=f"wm2t{h}")
                nc.sync.dma_start(wm2_t[h][:], wm2_sec[128 * h:128 * (h + 1), :])
            b_t = {n: load(n, [128, F], f32) for n in ("b0r", "b1r", "b2r")}
            bm1c_t = load("bm1c", [128, 4], f32)
            bm2r_t = load("bm2r", [128, NO], f32)
            iota_t = load("iota", [128, G], f16)
            bcol_t = load("bcol", [128, TILES], f32)
            invc_t = load("invc", [128, TILES], f32)

            # ---- stage x shard -> AllGather into the shared feature table ----
            xstage = res.tile([128, TILES * F], f16, tag="xstage")
            nc.sync.dma_start(xstage[:], sec("xsh"))
            nc.sync.dma_start(
                shard_d.ap().rearrange("(t p) f -> p t f", p=128),
                xstage[:].rearrange("p (t f) -> p t f", f=F))
            if variant not in ("nocc", "nogather_nocc"):
                nc.gpsimd.collective_compute(
                    "AllGather", mybir.AluOpType.bypass,
                    replica_groups=[list(range(NCORES))],
                    ins=[shard_d[:].opt()], outs=[tabn[:].opt()])

            layer_w = [("w0", "b0r", True), ("w1", "b1r", True), ("w2", "b2r", False)]

            if variant == "cconly":
                nc.vector.memset(shb[:, :], 0.0)
            for l in range(3):
                if variant == "cconly":
                    if l < 2:
                        nc.sync.dma_start(
                            shard_d.ap().rearrange("(t p) f -> p t f", p=128),
                            shb[:, :].rearrange("p (t f) -> p t f", f=F))
                        nc.gpsimd.collective_compute(
                            "AllGather", mybir.AluOpType.bypass,
                            replica_groups=[list(range(NCORES))],
                            ins=[shard_d[:].opt()], outs=[tabn[:].opt()])
                    continue
                tbl_ap = [tabn[0:LO, :], tabn[LO:NP, :] if HI > 0 else None]
                wname, bname, relu = layer_w[l]
                issued = [-1, -1]        # last issued batch per class
                cur = [None, None]       # current msg tile per class
                g = 0
                for w in range(NWIN):
                    width = min(WINW, SH - w * WINW)
                    chunks = schedule[w]
                    if variant not in ("gatheronly", "gs", "loadonly"):
                        ps = wps.tile([128, WINW], f32, tag="wps", name="ps")
                    else:
                        ps = None
                    for j, (cls, cid) in enumerate(chunks):
                        b, slab = divmod(cid, BATCH_CH)
                        if b != issued[cls] and variant != "loadonly":
                            nb = min(BATCH_CH, nch_cls[cls] - b * BATCH_CH)
                            mt = msgp.tile([128, BATCH_CH, F], f16, tag=f"msg{cls}")
                            if variant == "memset":
                                nc.vector.memset(mt[:, :nb, :], 0.0)
                            elif variant not in ("nogather", "nogather_nocc",
                                                 "loadonly"):
                                nc.gpsimd.dma_gather(
                                    mt[:, :nb, :], tbl_ap[cls],
                                    idx_t[cls][:, b * (BATCH_CH * 8):
                                               b * (BATCH_CH * 8) + nb * 8],
                                    nb * CH, nb * CH, F, single_packet=False)
                            issued[cls] = b
                            cur[cls] = mt
                        if variant in ("gatheronly", "loadonly"):
                            g += 1
                            continue
                        S = sp.tile([128, WINW], f16, tag="S")
                        nc.vector.tensor_scalar(
                            out=S[:, :width], in0=iota_t[:, :width],
                            scalar1=meta_t[:, 2 * g:2 * g + 1],
                            scalar2=meta_t[:, 2 * g + 1:2 * g + 2],
                            op0=mybir.AluOpType.is_equal,
                            op1=mybir.AluOpType.mult)
                        if variant == "gs":
                            g += 1
                            continue
                        nc.tensor.matmul(
                            out=ps[:, :width], lhsT=cur[cls][:, slab, :],
                            rhs=S[:, :width],
                            start=(j == 0), stop=(j == len(chunks) - 1))
                        g += 1
                    if variant in ("gatheronly", "gs", "loadonly"):
                        continue
                    aggT = aggp.tile([128, WINW], f16, tag="aggT")
                    nc.vector.tensor_copy(aggT[:, :width], ps[:, :width])
                    if variant == "gsm":
                        continue
                    for sub in range(width // 128):
                        t_idx = w * (WINW // 128) + sub
                        hp = hps.tile([128, F], f32, tag="hp")
                        nc.tensor.matmul(
                            out=hp[:], lhsT=aggT[:, sub * 128:(sub + 1) * 128],
                            rhs=w_t[wname][:], start=True, stop=True)
                        tmp = tmpp.tile([128, F], f32, tag="htmp")
                        nc.vector.tensor_tensor(
                            out=tmp[:], in0=hp[:], in1=b_t[bname][:],
                            op=mybir.AluOpType.add)
                        dst_sl = shb[:, t_idx * F:(t_idx + 1) * F]
                        if relu:
                            nc.vector.tensor_scalar(
                                out=dst_sl, in0=tmp[:], scalar1=0.0, scalar2=None,
                                op0=mybir.AluOpType.max)
                        else:
                            nc.vector.tensor_copy(dst_sl, tmp[:])
                if variant in ("gatheronly", "gs", "gsm", "loadonly"):
                    continue
                assert g == ncht
                if l < 2:
                    nc.sync.dma_start(
                        shard_d.ap().rearrange("(t p) f -> p t f", p=128),
                        shb[:, :].rearrange("p (t f) -> p t f", f=F))
                    if variant not in ("nocc", "nogather_nocc"):
                        nc.gpsimd.collective_compute(
                            "AllGather", mybir.AluOpType.bypass,
                            replica_groups=[list(range(NCORES))],
                            ins=[shard_d[:].opt()], outs=[tabn[:].opt()])

            # ---- mean pool ----
            if variant in ("gatheronly", "gs", "gsm", "loadonly"):
                if rep < repeat - 1:
                    continue
                z = tmpp.tile([128, NO], f32, tag="ot", name="zot")
                nc.vector.memset(z[:], 0.0)
                nc.vector.tensor_copy(shb[:, 0:NO], z[:])
                for gh in range(G // 128):
                    nc.sync.dma_start(out[128 * gh:128 * (gh + 1), :], z[:])
                raise _SkipRest
            gp = gps.tile([128, G], f32, tag="gp")
            for t in range(TILES):
                Gt = sp.tile([128, G], f16, tag="S")
                nc.vector.tensor_scalar(
                    out=Gt[:], in0=iota_t[:],
                    scalar1=bcol_t[:, t:t + 1], scalar2=invc_t[:, t:t + 1],
                    op0=mybir.AluOpType.is_equal, op1=mybir.AluOpType.mult)
                nc.tensor.matmul(out=gp[:], lhsT=shb[:, t * F:(t + 1) * F],
                                 rhs=Gt[:], start=(t == 0), stop=(t == TILES - 1))
            gtile = tmpp.tile([128, G], f32, tag="gtile")
            nc.vector.tensor_copy(gtile[:], gp[:])
            nc.sync.dma_start(gt_in[:], gtile[:])
            if variant not in ("nocc", "nogather_nocc"):
                nc.gpsimd.collective_compute(
                    "AllReduce", mybir.AluOpType.add,
                    replica_groups=[list(range(NCORES))],
                    ins=[gt_in[:].opt()], outs=[gt_out[:].opt()])
            gt16 = tmpp.tile([128, G], f16, tag="gt16")
            gfull = tmpp.tile([128, G], f32, tag="gfull")
            nc.sync.dma_start(gfull[:], gt_out[:])
            nc.vector.tensor_copy(gt16[:], gfull[:])

            # ---- MLP ----
            mt16 = []
            for h in range(4):
                mp = mps.tile([128, G], f32, tag="mp")
                nc.tensor.matmul(out=mp[:], lhsT=wm1_t[:, 128 * h:128 * (h + 1)],
                                 rhs=gt16[:], start=True, stop=True)
                mtile = tmpp.tile([128, G], f16, tag=f"mt{h}", name=f"mtile{h}")
                nc.vector.tensor_scalar(
                    out=mtile[:], in0=mp[:], scalar1=bm1c_t[:, h:h + 1],
                    scalar2=0.0, op0=mybir.AluOpType.add, op1=mybir.AluOpType.max)
                mt16.append(mtile)
            for gh in range(G // 128):
                op = mps.tile([128, NO], f32, tag="mp", name="op")
                for h in range(4):
                    nc.tensor.matmul(
                        out=op[:], lhsT=mt16[h][:, 128 * gh:128 * (gh + 1)],
                        rhs=wm2_t[h][:], start=(h == 0), stop=(h == 3))
                ot = tmpp.tile([128, NO], f32, tag="ot")
                nc.vector.tensor_tensor(out=ot[:], in0=op[:], in1=bm2r_t[:],
                                        op=mybir.AluOpType.add)
                nc.sync.dma_start(out[128 * gh:128 * (gh + 1), :], ot[:])

    nc.compile()
    return nc


def _get_built(inputs):
    import hashlib
    h = hashlib.sha1()
    for k in sorted(inputs):
        h.update(k.encode())
        h.update(np.ascontiguousarray(inputs[k]).tobytes())
    key = h.hexdigest()
    if key not in _cache:
        geom, in_maps = _host_prep(**inputs)
        nc = _build_bass(geom)
        _cache[key] = (geom, nc, in_maps)
    return _cache[key]


def kernel(**inputs):
    inputs = {k: np.asarray(v) for k, v in inputs.items()}
    geom, nc, in_maps = _get_built(inputs)
    from concourse.bass_utils import run_bass_kernel_spmd
    res = run_bass_kernel_spmd(nc, in_maps, list(range(NCORES)))
    return np.asarray(res.results[0]["out"])


# revision 17
# speedup vs baseline: 12.4017x; 1.3573x over previous
"""GCN encoder (3x GCNConv + mean-pool + MLP) as an 8-core Trainium2 Bass kernel.

Sharding: nodes/edges partitioned by destination-node owner (8 shards).
All per-core constant data (x shard, edge-index streams, per-chunk metadata,
weights, pooling helpers) is packed host-side into ONE int16 DRAM blob per
core — per-execution overhead in this environment scales with the number of
bound tensors, so the kernel binds exactly one input and one output.

Per execution: the x shard is staged to DRAM and AllGather'd into a
replicated [NP, F] f16 feature table.  Per layer: per-edge source features
are fetched with dma_gather, scaled+scattered into per-destination-window
sums via a PE matmul against a one-hot selection matrix built on DVE, then
the layer weight matmul + bias + ReLU produces this core's shard of the
next layer's features, which an AllGather replicates into every core's
table.  Mean-pool is a matmul against a per-graph one-hot (scaled by
1/count), AllReduce-summed across cores; the tiny MLP runs replicated.
"""

import numpy as np

NCORES = 8
F = 128            # hidden width (all layers padded to this)
G = 256            # number of graphs
NH = 512           # MLP hidden
NO = 256           # MLP out
CH = 128           # edges per chunk
BATCH_CH = 32      # chunks per dma_gather batch
WINW = 256         # dst nodes per PSUM accumulation window

_cache = {}


def _host_prep(x, edge_index, batch, W0, b0, W1, b1, W2, b2, Wm1, bm1, Wm2, bm2):
    N = x.shape[0]
    FI = x.shape[1]
    SH = -(-N // (NCORES * 128)) * 128      # shard size (nodes), 128-multiple
    NP = SH * NCORES
    TILES = SH // 128
    NWIN = -(-SH // WINW)
    LO = min(32768, NP)
    HI = NP - LO

    src = np.concatenate([edge_index[0], np.arange(N, dtype=np.int64)])
    dst = np.concatenate([edge_index[1], np.arange(N, dtype=np.int64)])
    deg = np.bincount(dst, minlength=N).astype(np.float32)
    dis = np.where(deg > 0, 1.0 / np.sqrt(np.maximum(deg, 1.0)), 0.0).astype(np.float32)
    norm = dis[src] * dis[dst]

    xpad = np.zeros((NP, F), dtype=np.float16)
    xpad[:N, :FI] = x.astype(np.float16)

    # per-core edge selection, ordered by (window, class, dst)
    per_core = []
    for c in range(NCORES):
        base = c * SH
        sel = (dst >= base) & (dst < base + SH)
        es = src[sel].astype(np.int64)
        ed = (dst[sel] - base).astype(np.int64)
        en = norm[sel]
        cl = (es >= LO).astype(np.int64)
        wi = ed // WINW
        order = np.lexsort((ed, cl, wi))
        per_core.append((es[order], ed[order], en[order], cl[order], wi[order]))

    # chunk counts per (window, class), equalized across cores
    counts = np.zeros((NCORES, NWIN, 2), dtype=np.int64)
    for c in range(NCORES):
        _, _, _, cl, wi = per_core[c]
        for cls in (0, 1):
            cnt = np.bincount(wi[cl == cls], minlength=NWIN)
            counts[c, :, cls] = cnt
    nch = -(-counts.max(axis=0) // CH)  # [NWIN, 2] chunks
    nch_cls = nch.sum(axis=0)          # total chunks per class
    ncht = int(nch.sum())

    # shared program schedule: windows -> list of (cls, cid); meta col = global g
    schedule = []
    cid_ctr = [0, 0]
    for w in range(NWIN):
        lst = []
        for cls in (0, 1):
            for _ in range(int(nch[w, cls])):
                lst.append((cls, cid_ctr[cls]))
                cid_ctr[cls] += 1
        schedule.append(lst)

    # per-core streams
    idx_streams = [[], []]   # per class: list over cores of [16, n/16] int16
    metas = []
    for c in range(NCORES):
        es, ed, en, cl, wi = per_core[c]
        idx_parts = [[], []]
        meta = np.zeros((128, 2 * ncht), dtype=np.float16)
        g = 0
        pos = 0
        # edges are sorted (win, cls, dst); walk groups in the same order
        for w in range(NWIN):
            for cls in (0, 1):
                n_e = int(counts[c, w, cls])
                tot = int(nch[w, cls]) * CH
                ge, gd, gn = es[pos:pos + n_e], ed[pos:pos + n_e], en[pos:pos + n_e]
                pos += n_e
                pad = tot - n_e
                iv = ge - (LO if cls else 0)
                iv = np.concatenate([iv, np.zeros(pad, np.int64)])
                dl = np.concatenate([gd - w * WINW, np.zeros(pad, np.int64)])
                nr = np.concatenate([gn, np.zeros(pad, np.float32)])
                idx_parts[cls].append(iv.astype(np.int16))
                for k in range(tot // CH):
                    meta[:, 2 * g] = dl[k * CH:(k + 1) * CH].astype(np.float16)
                    meta[:, 2 * g + 1] = nr[k * CH:(k + 1) * CH].astype(np.float16)
                    g += 1
        assert g == ncht
        for cls in (0, 1):
            arr = (np.concatenate(idx_parts[cls]) if idx_parts[cls]
                   else np.zeros(0, np.int16))
            assert arr.size == nch_cls[cls] * CH
            if arr.size:
                wrapped = np.ascontiguousarray(arr.reshape(-1, 16).T)  # [16, n/16]
            else:
                wrapped = np.zeros((16, 8), np.int16)  # dummy
            idx_streams[cls].append(wrapped)
        metas.append(meta)

    # pooling helpers
    cnt = np.bincount(batch.astype(np.int64), minlength=G).astype(np.float32)
    invc_all = (1.0 / np.maximum(cnt, 1.0))[batch.astype(np.int64)]
    bcols, invcs = [], []
    for c in range(NCORES):
        sl = slice(c * SH, min((c + 1) * SH, N))
        b_sh = np.zeros(SH, np.float32)
        i_sh = np.zeros(SH, np.float32)
        nreal = max(0, min((c + 1) * SH, N) - c * SH)
        if nreal > 0:
            b_sh[:nreal] = batch[sl].astype(np.float32)
            i_sh[:nreal] = invc_all[sl].astype(np.float32)
        bcols.append(np.ascontiguousarray(b_sh.reshape(TILES, 128).T))  # [128,TILES]
        invcs.append(np.ascontiguousarray(i_sh.reshape(TILES, 128).T))

    W0p = np.zeros((F, F), np.float16)
    W0p[:FI] = W0.astype(np.float16)

    # ---- blob packing ----
    # Section table: name -> (shape, dtype).  All cores share shapes; offsets
    # are in int16 elements, 128-aligned.
    n0 = max(int(nch_cls[0]) * 8, 8)
    n1 = max(int(nch_cls[1]) * 8, 8)
    sections = [
        ("xsh", (128, TILES * F), np.float16),
        ("idxlo", (16, n0), np.int16),
        ("idxhi", (16, n1), np.int16),
        ("meta", (128, 2 * ncht), np.float16),
        ("w0", (F, F), np.float16),
        ("w1", (F, F), np.float16),
        ("w2", (F, F), np.float16),
        ("wm1", (F, NH), np.float16),
        ("wm2", (NH, NO), np.float16),
        ("iota", (128, G), np.float16),
        ("b0r", (128, F), np.float32),
        ("b1r", (128, F), np.float32),
        ("b2r", (128, F), np.float32),
        ("bm1c", (128, 4), np.float32),
        ("bm2r", (128, NO), np.float32),
        ("bcol", (128, TILES), np.float16),
        ("invc", (128, TILES), np.float16),
    ]
    offs = {}
    pos16 = 0
    for name, shape, dtype in sections:
        n16 = int(np.prod(shape)) * np.dtype(dtype).itemsize // 2
        offs[name] = (pos16, shape, dtype)
        pos16 += -(-n16 // 128) * 128
    TOT = pos16

    consts = {
        "w0": W0p, "w1": W1.astype(np.float16), "w2": W2.astype(np.float16),
        "wm1": Wm1.astype(np.float16), "wm2": Wm2.astype(np.float16),
        "b0r": np.tile(b0.astype(np.float32)[None, :], (128, 1)),
        "b1r": np.tile(b1.astype(np.float32)[None, :], (128, 1)),
        "b2r": np.tile(b2.astype(np.float32)[None, :], (128, 1)),
        "bm1c": np.ascontiguousarray(bm1.astype(np.float32).reshape(4, 128).T),
        "bm2r": np.tile(bm2.astype(np.float32)[None, :], (128, 1)),
        "iota": np.tile(np.arange(G, dtype=np.float16)[None, :], (128, 1)),
    }
    in_maps = []
    for c in range(NCORES):
        blob = np.zeros(TOT, np.int16)

        def put(name, arr):
            off, shape, dtype = offs[name]
            a = np.ascontiguousarray(arr.astype(dtype, copy=False))
            assert a.shape == shape, (name, a.shape, shape)
            v = a.view(np.int16).reshape(-1)
            blob[off:off + v.size] = v

        xstage = (xpad[c * SH:(c + 1) * SH]
                  .reshape(TILES, 128, F).transpose(1, 0, 2).reshape(128, TILES * F))
        put("xsh", xstage)
        ilo = idx_streams[0][c]
        ihi = idx_streams[1][c]
        ilo_p = np.zeros((16, n0), np.int16); ilo_p[:, :ilo.shape[1]] = ilo
        ihi_p = np.zeros((16, n1), np.int16); ihi_p[:, :ihi.shape[1]] = ihi
        put("idxlo", ilo_p)
        put("idxhi", ihi_p)
        put("meta", metas[c])
        for k, v in consts.items():
            put(k, v)
        put("bcol", bcols[c])
        put("invc", invcs[c])
        in_maps.append({"blob": blob.reshape(1, TOT)})

    geom = dict(N=N, NP=NP, SH=SH, TILES=TILES, NWIN=NWIN, LO=LO, HI=HI,
                nch=nch, nch_cls=[int(v) for v in nch_cls], ncht=ncht,
                schedule=schedule, offs=offs, TOT=TOT, n0=n0, n1=n1,
                smw=max(len(s) for s in schedule))
    return geom, in_maps


class _SkipRest(Exception):
    pass


def _build_bass(geom, variant="full", repeat=1, swq=4, spkt=False,
                batch_ch=BATCH_CH, msg_bufs=3):
    import concourse.bass as bass
    import concourse.tile as tile
    from concourse import bacc, mybir

    f16, f32, i16 = mybir.dt.float16, mybir.dt.float32, mybir.dt.int16
    dt_map = {np.float16: f16, np.float32: f32, np.int16: i16}
    NP, SH, TILES, NWIN = geom["NP"], geom["SH"], geom["TILES"], geom["NWIN"]
    LO, HI = geom["LO"], geom["HI"]
    nch, nch_cls, ncht = geom["nch"], geom["nch_cls"], geom["ncht"]
    schedule, offs, TOT = geom["schedule"], geom["offs"], geom["TOT"]
    n0, n1 = geom["n0"], geom["n1"]
    SMW = geom["smw"]

    nc = bacc.Bacc("TRN2", target_bir_lowering=False, debug=False,
                   num_devices=NCORES, num_swdge_queues=swq)

    blob = nc.dram_tensor("blob", [1, TOT], i16, kind="ExternalInput")
    out = nc.dram_tensor("out", [G, NO], f32, kind="ExternalOutput")

    shard_d = nc.dram_tensor("shard_d", [SH, F], f16)
    tabn = nc.dram_tensor("tabn", [NP, F], f16, addr_space="Shared")
    gt_in = nc.dram_tensor("gt_in", [128, G], f32)
    gt_out = nc.dram_tensor("gt_out", [128, G], f32, addr_space="Shared")

    shb = nc.alloc_sbuf_tensor("shb", [128, TILES * F], f16)

    def sec(name):
        off, shape, dtype = offs[name]
        nel = int(np.prod(shape))
        bdt = dt_map[dtype]
        n16 = nel * np.dtype(dtype).itemsize // 2
        ap = blob[0, off:off + n16].bitcast(bdt)
        return ap.rearrange("(p n) -> p n", p=shape[0])

    import contextlib
    with tile.TileContext(nc) as tc:
        with (
            contextlib.suppress(_SkipRest),
            tc.tile_pool(name="res", bufs=1) as res,
            tc.tile_pool(name="msg", bufs=msg_bufs) as msgp,
            tc.tile_pool(name="sp", bufs=2) as sp,
            tc.tile_pool(name="agg", bufs=2) as aggp,
            tc.tile_pool(name="tmp", bufs=2) as tmpp,
            tc.tile_pool(name="wps", bufs=2, space="PSUM") as wps,
            tc.tile_pool(name="hps", bufs=2, space="PSUM") as hps,
            tc.tile_pool(name="gps", bufs=1, space="PSUM") as gps,
            tc.tile_pool(name="mps", bufs=1, space="PSUM") as mps,
        ):
          for rep in range(repeat):
            # ---- resident loads (all from the blob) ----
            def load(name, shape, dtype):
                t = res.tile(list(shape), dtype, tag=name)
                nc.sync.dma_start(t[:], sec(name))
                return t

            idx_t = []
            for cls, nn in ((0, n0), (1, n1)):
                t = res.tile([128, nn], i16, tag=f"idx{cls}")
                src_ap = sec(f"idx{'lo' if cls == 0 else 'hi'}")
                for k in range(8):
                    nc.sync.dma_start(t[16 * k:16 * (k + 1), :], src_ap)
                idx_t.append(t)
            meta_t = load("meta", [128, 2 * ncht], f16)
            w_t = {n: load(n, [F, F], f16) for n in ("w0", "w1", "w2")}
            wm1_t = load("wm1", [F, NH], f16)
            wm2_sec = sec("wm2")
            wm2_t = [None] * 4
            for h in range(4):
                wm2_t[h] = res.tile([128, NO], f16, tag=f"wm2_{h}", name=f"wm2t{h}")
                nc.sync.dma_start(wm2_t[h][:], wm2_sec[128 * h:128 * (h + 1), :])
            b_t = {n: load(n, [128, F], f32) for n in ("b0r", "b1r", "b2r")}
            bm1c_t = load("bm1c", [128, 4], f32)
            bm2r_t = load("bm2r", [128, NO], f32)
            iota_t = load("iota", [128, G], f16)
            bcol_t = load("bcol", [128, TILES], f16)
            invc_t = load("invc", [128, TILES], f16)

            # ---- stage x shard -> AllGather into the shared feature table ----
            xstage = res.tile([128, TILES * F], f16, tag="xstage")
            nc.sync.dma_start(xstage[:], sec("xsh"))
            nc.sync.dma_start(
                shard_d.ap().rearrange("(t p) f -> p t f", p=128),
                xstage[:].rearrange("p (t f) -> p t f", f=F))
            if variant not in ("nocc", "nogather_nocc"):
                nc.gpsimd.collective_compute(
                    "AllGather", mybir.AluOpType.bypass,
                    replica_groups=[list(range(NCORES))],
                    ins=[shard_d[:].opt()], outs=[tabn[:].opt()])

            layer_w = [("w0", "b0r", True), ("w1", "b1r", True), ("w2", "b2r", False)]

            if variant == "cconly":
                nc.vector.memset(shb[:, :], 0.0)
            for l in range(3):
                if variant == "cconly":
                    if l < 2:
                        nc.sync.dma_start(
                            shard_d.ap().rearrange("(t p) f -> p t f", p=128),
                            shb[:, :].rearrange("p (t f) -> p t f", f=F))
                        nc.gpsimd.collective_compute(
                            "AllGather", mybir.AluOpType.bypass,
                            replica_groups=[list(range(NCORES))],
                            ins=[shard_d[:].opt()], outs=[tabn[:].opt()])
                    continue
                tbl_ap = [tabn[0:LO, :], tabn[LO:NP, :] if HI > 0 else None]
                wname, bname, relu = layer_w[l]
                gq = [0]
                issued = [-1, -1]        # last issued batch per class
                cur = [None, None]       # current msg tile per class
                g = 0
                for w in range(NWIN):
                    width = min(WINW, SH - w * WINW)
                    chunks = schedule[w]
                    nchw = len(chunks)
                    if variant not in ("gatheronly", "gs", "loadonly"):
                        ps = wps.tile([128, WINW], f32, tag="wps", name="ps")
                    else:
                        ps = None
                    if variant not in ("gatheronly", "loadonly"):
                        # batched one-hot build: S[p,k,c] =
                        #   (iota[p,c] == dl[p,k]) * nr[p,k]  for the whole window
                        St = sp.tile([128, SMW, WINW], f16, tag="S", name="St")
                        io_ap = (iota_t[:, :width]
                                 .rearrange("p (k c) -> p k c", k=1)
                                 .broadcast_to([128, nchw, width]))
                        m2 = (meta_t[:, 2 * g:2 * (g + nchw)]
                              .rearrange("p (k two) -> p k two", two=2))
                        dl_ap = m2[:, :, 0:1].broadcast_to([128, nchw, width])
                        nr_ap = m2[:, :, 1:2].broadcast_to([128, nchw, width])
                        nc.vector.tensor_tensor(
                            out=St[:, :nchw, :width], in0=io_ap, in1=dl_ap,
                            op=mybir.AluOpType.is_equal)
                        nc.vector.tensor_tensor(
                            out=St[:, :nchw, :width], in0=St[:, :nchw, :width],
                            in1=nr_ap, op=mybir.AluOpType.mult)
                    for j, (cls, cid) in enumerate(chunks):
                        b, slab = divmod(cid, batch_ch)
                        if b != issued[cls] and variant != "loadonly":
                            nb = min(batch_ch, nch_cls[cls] - b * batch_ch)
                            mt = msgp.tile([128, batch_ch, F], f16, tag=f"msg{cls}")
                            if variant == "memset":
                                nc.vector.memset(mt[:, :nb, :], 0.0)
                            elif variant not in ("nogather", "nogather_nocc",
                                                 "loadonly"):
                                nc.gpsimd.dma_gather(
                                    mt[:, :nb, :], tbl_ap[cls],
                                    idx_t[cls][:, b * (batch_ch * 8):
                                               b * (batch_ch * 8) + nb * 8],
                                    nb * CH, nb * CH, F, single_packet=spkt,
                                    queue_num=gq[0] % swq)
                                gq[0] += 1
                            issued[cls] = b
                            cur[cls] = mt
                        if variant in ("gatheronly", "loadonly", "gs"):
                            g += 1
                            continue
                        nc.tensor.matmul(
                            out=ps[:, :width], lhsT=cur[cls][:, slab, :],
                            rhs=St[:, j, :width],
                            start=(j == 0), stop=(j == len(chunks) - 1))
                        g += 1
                    if variant in ("gatheronly", "gs", "loadonly"):
                        continue
                    aggT = aggp.tile([128, WINW], f16, tag="aggT")
                    nc.vector.tensor_copy(aggT[:, :width], ps[:, :width])
                    if variant == "gsm":
                        continue
                    for sub in range(width // 128):
                        t_idx = w * (WINW // 128) + sub
                        hp = hps.tile([128, F], f32, tag="hp")
                        nc.tensor.matmul(
                            out=hp[:], lhsT=aggT[:, sub * 128:(sub + 1) * 128],
                            rhs=w_t[wname][:], start=True, stop=True)
                        tmp = tmpp.tile([128, F], f32, tag="htmp")
                        nc.vector.tensor_tensor(
                            out=tmp[:], in0=hp[:], in1=b_t[bname][:],
                            op=mybir.AluOpType.add)
                        dst_sl = shb[:, t_idx * F:(t_idx + 1) * F]
                        if relu:
                            nc.vector.tensor_scalar(
                                out=dst_sl, in0=tmp[:], scalar1=0.0, scalar2=None,
                                op0=mybir.AluOpType.max)
                        else:
                            nc.vector.tensor_copy(dst_sl, tmp[:])
                if variant in ("gatheronly", "gs", "gsm", "loadonly"):
                    continue
                assert g == ncht
                if l < 2:
                    nc.sync.dma_start(
                        shard_d.ap().rearrange("(t p) f -> p t f", p=128),
                        shb[:, :].rearrange("p (t f) -> p t f", f=F))
                    if variant not in ("nocc", "nogather_nocc"):
                        nc.gpsimd.collective_compute(
                            "AllGather", mybir.AluOpType.bypass,
                            replica_groups=[list(range(NCORES))],
                            ins=[shard_d[:].opt()], outs=[tabn[:].opt()])

            # ---- mean pool ----
            if variant in ("gatheronly", "gs", "gsm", "loadonly"):
                if rep < repeat - 1:
                    continue
                z = tmpp.tile([128, NO], f32, tag="ot", name="zot")
                nc.vector.memset(z[:], 0.0)
                nc.vector.tensor_copy(shb[:, 0:NO], z[:])
                for gh in range(G // 128):
                    nc.sync.dma_start(out[128 * gh:128 * (gh + 1), :], z[:])
                raise _SkipRest
            gp = gps.tile([128, G], f32, tag="gp")
            Gt = sp.tile([128, TILES, G], f16, tag="S", name="Gt")
            io_g = (iota_t[:].rearrange("p (k c) -> p k c", k=1)
                    .broadcast_to([128, TILES, G]))
            bc_ap = (bcol_t[:].rearrange("p (k one) -> p k one", one=1)
                     .broadcast_to([128, TILES, G]))
            iv_ap = (invc_t[:].rearrange("p (k one) -> p k one", one=1)
                     .broadcast_to([128, TILES, G]))
            nc.vector.tensor_tensor(out=Gt[:], in0=io_g, in1=bc_ap,
                                    op=mybir.AluOpType.is_equal)
            nc.vector.tensor_tensor(out=Gt[:], in0=Gt[:], in1=iv_ap,
                                    op=mybir.AluOpType.mult)
            for t in range(TILES):
                nc.tensor.matmul(out=gp[:], lhsT=shb[:, t * F:(t + 1) * F],
                                 rhs=Gt[:, t, :], start=(t == 0),
                                 stop=(t == TILES - 1))
            gtile = tmpp.tile([128, G], f32, tag="gtile")
            nc.vector.tensor_copy(gtile[:], gp[:])
            nc.sync.dma_start(gt_in[:], gtile[:])
            if variant not in ("nocc", "nogather_nocc"):
                nc.gpsimd.collective_compute(
                    "AllReduce", mybir.AluOpType.add,
                    replica_groups=[list(range(NCORES))],
                    ins=[gt_in[:].opt()], outs=[gt_out[:].opt()])
            gt16 = tmpp.tile([128, G], f16, tag="gt16")
            gfull = tmpp.tile([128, G], f32, tag="gfull")
            nc.sync.dma_start(gfull[:], gt_out[:])
            nc.vector.tensor_copy(gt16[:], gfull[:])

            # ---- MLP ----
            mt16 = []
            for h in range(4):
                mp = mps.tile([128, G], f32, tag="mp")
                nc.tensor.matmul(out=mp[:], lhsT=wm1_t[:, 128 * h:128 * (h + 1)],
                                 rhs=gt16[:], start=True, stop=True)
                mtile = tmpp.tile([128, G], f16, tag=f"mt{h}", name=f"mtile{h}")
                nc.vector.tensor_scalar(
                    out=mtile[:], in0=mp[:], scalar1=bm1c_t[:, h:h + 1],
                    scalar2=0.0, op0=mybir.AluOpType.add, op1=mybir.AluOpType.max)
                mt16.append(mtile)
            for gh in range(G // 128):
                op = mps.tile([128, NO], f32, tag="mp", name="op")
                for h in range(4):
                    nc.tensor.matmul(
                        out=op[:], lhsT=mt16[h][:, 128 * gh:128 * (gh + 1)],
                        rhs=wm2_t[h][:], start=(h == 0), stop=(h == 3))
                ot = tmpp.tile([128, NO], f32, tag="ot")
                nc.vector.tensor_tensor(out=ot[:], in0=op[:], in1=bm2r_t[:],
                                        op=mybir.AluOpType.add)
                nc.sync.dma_start(out[128 * gh:128 * (gh + 1), :], ot[:])

    nc.compile()
    return nc


def _get_built(inputs):
    import hashlib
    h = hashlib.sha1()
    for k in sorted(inputs):
        h.update(k.encode())
        h.update(np.ascontiguousarray(inputs[k]).tobytes())
    key = h.hexdigest()
    if key not in _cache:
        geom, in_maps = _host_prep(**inputs)
        nc = _build_bass(geom)
        _cache[key] = (geom, nc, in_maps)
    return _cache[key]


def kernel(**inputs):
    inputs = {k: np.asarray(v) for k, v in inputs.items()}
    geom, nc, in_maps = _get_built(inputs)
    from concourse.bass_utils import run_bass_kernel_spmd
    res = run_bass_kernel_spmd(nc, in_maps, list(range(NCORES)))
    return np.asarray(res.results[0]["out"])
